# revision 2
# baseline (speedup 1.0000x reference)
"""Trainium2 Bass kernel for nn_Net_1975684956439 (scatter_memory).

Computation (reference):
  e_pa  = sum(coeffs * weight1) / num_atoms + bias1                  # [1]
  f     = -sum(coeffs_derivs * weight1, axis=3)                      # [1, 3, P]
  out_f = segment_sum(f[0].T, neigh_atom_index, num_atoms).T[None]   # [1, 3, N]

Strategy: data-parallel over the pair axis P=131072 across 8 NeuronCores
(16384 pairs/core).  Per core:
  - DMA coeffs_derivs tiles [128 parts, 2048] fp32 (1 MB each, 12 per core);
    partition p holds 32 consecutive pairs (32*64 descriptors).
  - ACT casts fp32 -> bf16.
  - DVE multiplies by -weight1 (broadcast along free dim).
  - GPSIMD does two halving adds (64 -> 16 per pair), DVE finishes the
    grouped reduce (16 -> 1) giving per-pair forces f [128, 32] per plane.
  - Scatter via one-hot matmul: atom n = 128*q + r.  DVE builds the r
    one-hot (is_equal vs iota) and g = f * qmask (q one-hot times force),
    TensorE accumulates psum[r, q*3+c] += onehot_r.T @ g over all pairs.
  - e_pa partial: trivial multiply+reduce of this core's coeffs slice.
Host sums the 8 per-core partials ([128,48] each) and rearranges.
"""

import numpy as np
import ml_dtypes

N_CORES = 8
NUM_ATOMS = 2048
ND = 64            # descriptors
P_TOT = 131072     # pairs
PLOC = P_TOT // N_CORES   # 16384 pairs per core
PPP = 32           # pairs per partition per block (= matmul groups per block)
NBLK = 4           # blocks per core: 4 * 128 * 32 = 16384
NJ = PPP
FREE = PPP * ND    # 2048 free elements per cd tile

_BF16 = ml_dtypes.bfloat16

_CACHE = {}


def _build_nc():
    import concourse.bacc as bacc
    import concourse.tile as tile
    from concourse import mybir

    BF = mybir.dt.bfloat16
    F32 = mybir.dt.float32
    OP = mybir.AluOpType
    AX = mybir.AxisListType.X
    ACTF = mybir.ActivationFunctionType

    nc = bacc.Bacc("TRN2", target_bir_lowering=False, debug=False,
                   num_devices=N_CORES)

    cd = nc.dram_tensor("cd", [3 * NBLK, 128, FREE], F32, kind="ExternalInput")
    ridx = nc.dram_tensor("ridx", [128, NBLK * NJ], F32, kind="ExternalInput")
    qidx = nc.dram_tensor("qidx", [128, NBLK * NJ], BF, kind="ExternalInput")
    io128 = nc.dram_tensor("io128", [128, 128], BF, kind="ExternalInput")
    io16 = nc.dram_tensor("io16", [128, 16], BF, kind="ExternalInput")
    wneg = nc.dram_tensor("wneg", [128, ND], BF, kind="ExternalInput")
    wpos = nc.dram_tensor("wpos", [128, ND], F32, kind="ExternalInput")
    coef = nc.dram_tensor("coef", [128, 2 * ND], F32, kind="ExternalInput")
    pf = nc.dram_tensor("pf", [128, 48], F32, kind="ExternalOutput")
    pe = nc.dram_tensor("pe", [128, 2], F32, kind="ExternalOutput")

    with tile.TileContext(nc) as tc:
        with nc.allow_low_precision("bf16 pipeline validated against fp32 reference"), \
             tc.tile_pool(name="singles", bufs=1) as singles, \
             tc.tile_pool(name="cdf", bufs=3) as cdf_pool, \
             tc.tile_pool(name="cdb", bufs=3) as cdb_pool, \
             tc.tile_pool(name="tmp", bufs=3) as tmp_pool, \
             tc.tile_pool(name="hh", bufs=3) as h_pool, \
             tc.tile_pool(name="ff", bufs=2) as f_pool, \
             tc.tile_pool(name="qg", bufs=2) as qg_pool, \
             tc.tile_pool(name="mm", bufs=2) as m_pool, \
             tc.tile_pool(name="psum", bufs=1, space="PSUM") as psum_pool:

            s_ridx = singles.tile([128, NBLK * NJ], F32)
            nc.sync.dma_start(out=s_ridx, in_=ridx[:, :])
            s_qidx = singles.tile([128, NBLK * NJ], BF)
            nc.sync.dma_start(out=s_qidx, in_=qidx[:, :])
            s_io128 = singles.tile([128, 128], BF)
            nc.sync.dma_start(out=s_io128, in_=io128[:, :])
            s_io16 = singles.tile([128, 16], BF)
            nc.sync.dma_start(out=s_io16, in_=io16[:, :])
            s_wneg = singles.tile([128, ND], BF)
            nc.sync.dma_start(out=s_wneg, in_=wneg[:, :])
            s_wpos = singles.tile([128, ND], F32)
            nc.sync.dma_start(out=s_wpos, in_=wpos[:, :])
            s_coef = singles.tile([128, 2 * ND], F32)
            nc.sync.dma_start(out=s_coef, in_=coef[:, :])

            # ---- e_pa partial: sum(coeffs * w) over this core's 256 atoms
            etmp = singles.tile([128, 2 * ND], F32)
            nc.vector.tensor_tensor(
                out=etmp[:].rearrange("p (a k) -> p a k", k=ND),
                in0=s_coef[:].rearrange("p (a k) -> p a k", k=ND),
                in1=s_wpos[:].unsqueeze(1).broadcast_to((128, 2, ND)),
                op=OP.mult)
            pe_sb = singles.tile([128, 2], F32)
            nc.vector.tensor_reduce(
                out=pe_sb, in_=etmp[:].rearrange("p (a k) -> p a k", k=ND),
                axis=AX, op=OP.add)
            nc.sync.dma_start(out=pe[:, :], in_=pe_sb)

            # ---- main pipeline
            pfp = psum_pool.tile([128, 48], F32)

            for b in range(NBLK):
                fb = f_pool.tile([128, NJ * 3], BF)
                for c in range(3):
                    i = c * NBLK + b
                    cdf = cdf_pool.tile([128, FREE], F32)
                    nc.sync.dma_start(out=cdf, in_=cd[i, :, :])
                    cdb = cdb_pool.tile([128, FREE], BF)
                    nc.scalar.copy(out=cdb, in_=cdf)          # ACT cast
                    tmp = tmp_pool.tile([128, FREE], BF)
                    nc.vector.tensor_tensor(                   # DVE: * (-w)
                        out=tmp[:].rearrange("p (j k) -> p j k", k=ND),
                        in0=cdb[:].rearrange("p (j k) -> p j k", k=ND),
                        in1=s_wneg[:].unsqueeze(1).broadcast_to((128, NJ, ND)),
                        op=OP.mult)
                    t3 = tmp[:].rearrange("p (j k) -> p j k", k=ND)
                    h1 = h_pool.tile([128, NJ * 32], BF, tag="h1")
                    h1v = h1[:].rearrange("p (j k) -> p j k", k=32)
                    nc.gpsimd.tensor_tensor(                   # GPSIMD: 64->32
                        out=h1v, in0=t3[:, :, 0:32], in1=t3[:, :, 32:64],
                        op=OP.add)
                    h2 = h_pool.tile([128, NJ * 16], BF, tag="h2")
                    h2v = h2[:].rearrange("p (j k) -> p j k", k=16)
                    h1r = h1[:].rearrange("p (j k) -> p j k", k=32)
                    nc.gpsimd.tensor_tensor(                   # GPSIMD: 32->16
                        out=h2v, in0=h1r[:, :, 0:16], in1=h1r[:, :, 16:32],
                        op=OP.add)
                    fbv = fb[:].rearrange("p (j c) -> p j c", c=3)[:, :, c]
                    nc.vector.tensor_reduce(                   # DVE: 16->1
                        out=fbv,
                        in_=h2[:].rearrange("p (j k) -> p j k", k=16),
                        axis=AX, op=OP.add)

                # qmask: (iota16 == q_idx), batched over the 32 groups
                qm = qg_pool.tile([128, NJ * 16], BF, tag="qm")
                nc.vector.tensor_tensor(
                    out=qm[:].rearrange("p (j q) -> p j q", q=16),
                    in0=s_io16[:].unsqueeze(1).broadcast_to((128, NJ, 16)),
                    in1=s_qidx[:, b * NJ:(b + 1) * NJ].unsqueeze(2)
                        .broadcast_to((128, NJ, 16)),
                    op=OP.is_equal)
                # g = qmask * f  (Khatri-Rao via stride-0 broadcasts)
                g = qg_pool.tile([128, NJ * 48], BF, tag="g")
                nc.vector.tensor_tensor(
                    out=g[:].rearrange("p (j q c) -> p j q c", q=16, c=3),
                    in0=qm[:].rearrange("p (j q) -> p j q", q=16)
                        .unsqueeze(3).broadcast_to((128, NJ, 16, 3)),
                    in1=fb[:].rearrange("p (j c) -> p j c", c=3)
                        .unsqueeze(2).broadcast_to((128, NJ, 16, 3)),
                    op=OP.mult)
                # match: per-group one-hot over r (DVE tensor_scalar, 4x bf16)
                mt = m_pool.tile([128, NJ * 128], BF)
                for j in range(NJ):
                    col = b * NJ + j
                    nc.vector.tensor_scalar(
                        mt[:, j * 128:(j + 1) * 128], s_io128[:, :],
                        s_ridx[:, col:col + 1], None, OP.is_equal)
                # scatter: psum[r, q*3+c] += onehot_r.T @ g
                for j in range(NJ):
                    nc.tensor.matmul(
                        pfp,
                        lhsT=mt[:, j * 128:(j + 1) * 128],
                        rhs=g[:, j * 48:(j + 1) * 48],
                        start=(b == 0 and j == 0),
                        stop=(b == NBLK - 1 and j == NJ - 1))

            pf_sb = singles.tile([128, 48], F32)
            nc.scalar.copy(out=pf_sb, in_=pfp)
            nc.sync.dma_start(out=pf[:, :], in_=pf_sb)

    nc.compile()
    return nc


def _get_nc():
    if "nc" not in _CACHE:
        _CACHE["nc"] = _build_nc()
    return _CACHE["nc"]


def _host_prep(inputs):
    coeffs = np.asarray(inputs["coeffs"])            # [1, 2048, 64] f32
    cd_full = np.asarray(inputs["coeffs_derivs"])    # [1, 3, P, 64] f32
    nei = np.asarray(inputs["neigh_atom_index"])     # [P] int32
    w = np.asarray(inputs["weight1"])                # [1, 64] f32

    io128 = np.ascontiguousarray(
        np.tile(np.arange(128, dtype=np.float32).astype(_BF16), (128, 1)))
    io16 = np.ascontiguousarray(
        np.tile(np.arange(16, dtype=np.float32).astype(_BF16), (128, 1)))
    wneg = np.ascontiguousarray(np.tile((-w[0]).astype(_BF16), (128, 1)))
    wpos = np.ascontiguousarray(np.tile(w[0].astype(np.float32), (128, 1)))

    atoms_per_core = NUM_ATOMS // N_CORES            # 256
    in_maps = []
    for m in range(N_CORES):
        sl = slice(m * PLOC, (m + 1) * PLOC)
        cdm = np.ascontiguousarray(cd_full[0, :, sl, :]).reshape(
            3 * NBLK, 128, FREE)
        nm = nei[sl].reshape(NBLK, 128, NJ)          # [b, part, j]
        r = np.ascontiguousarray(
            (nm % 128).astype(np.float32)
            .transpose(1, 0, 2).reshape(128, NBLK * NJ))
        q = np.ascontiguousarray(
            (nm // 128).astype(np.float32).astype(_BF16)
            .transpose(1, 0, 2).reshape(128, NBLK * NJ))
        cf = np.ascontiguousarray(
            coeffs[0, m * atoms_per_core:(m + 1) * atoms_per_core, :]
            .reshape(2, 128, ND).transpose(1, 0, 2).reshape(128, 2 * ND))
        in_maps.append(dict(cd=cdm, ridx=r, qidx=q, io128=io128, io16=io16,
                            wneg=wneg, wpos=wpos, coef=cf))
    return in_maps


def run(inputs, trace=False, trace_kwargs=None):
    """Run the kernel; returns ((e_pa, out_f), BassKernelResults)."""
    from concourse.bass_utils import run_bass_kernel_spmd

    nc = _get_nc()
    in_maps = _host_prep(inputs)
    res = run_bass_kernel_spmd(
        nc, in_maps, core_ids=list(range(N_CORES)), trace=trace,
        **(trace_kwargs or {}))

    pf = np.zeros((128, 48), np.float64)
    pe_total = 0.0
    for i in range(N_CORES):
        pf += res.results[i]["pf"].astype(np.float64)
        pe_total += float(res.results[i]["pe"].sum(dtype=np.float64))

    out_f = np.ascontiguousarray(
        pf.reshape(128, 16, 3).transpose(2, 1, 0).reshape(1, 3, NUM_ATOMS)
        .astype(np.float32))
    bias1 = np.asarray(inputs["bias1"]).astype(np.float32)
    e_pa = (np.array([pe_total / NUM_ATOMS], np.float32) + bias1).astype(
        np.float32)
    return (e_pa, out_f), res


def kernel(**inputs):
    (e_pa, out_f), _ = run(inputs, trace=False)
    return (e_pa, out_f)


# revision 3
# speedup vs baseline: 1.0666x; 1.0666x over previous
"""Trainium2 Bass kernel for nn_Net_1975684956439 (scatter_memory).

Computation (reference):
  e_pa  = sum(coeffs * weight1) / num_atoms + bias1                  # [1]
  f     = -sum(coeffs_derivs * weight1, axis=3)                      # [1, 3, P]
  out_f = segment_sum(f[0].T, neigh_atom_index, num_atoms).T[None]   # [1, 3, N]

Strategy: data-parallel over the pair axis P=131072 across 8 NeuronCores
(16384 pairs/core).  Per core:
  - DMA coeffs_derivs tiles [128 parts, 2048] fp32 (1 MB each, 12 per core);
    partition p holds 32 consecutive pairs (32*64 descriptors).
  - ACT casts fp32 -> bf16.
  - DVE multiplies by -weight1 (broadcast along free dim).
  - GPSIMD does two halving adds (64 -> 16 per pair), DVE finishes the
    grouped reduce (16 -> 1) giving per-pair forces f [128, 32] per plane.
  - Scatter via one-hot matmul: atom n = 128*q + r.  DVE builds the r
    one-hot (is_equal vs iota) and g = f * qmask (q one-hot times force),
    TensorE accumulates psum[r, q*3+c] += onehot_r.T @ g over all pairs.
  - e_pa partial: trivial multiply+reduce of this core's coeffs slice.
Host sums the 8 per-core partials ([128,48] each) and rearranges.
"""

import numpy as np
import ml_dtypes

N_CORES = 8
NUM_ATOMS = 2048
ND = 64            # descriptors
P_TOT = 131072     # pairs
PLOC = P_TOT // N_CORES   # 16384 pairs per core
PPP = 32           # pairs per partition per block (= matmul groups per block)
NBLK = 4           # blocks per core: 4 * 128 * 32 = 16384
NJ = PPP
FREE = PPP * ND    # 2048 free elements per cd tile

_BF16 = ml_dtypes.bfloat16

_CACHE = {}


def _build_nc():
    import concourse.bacc as bacc
    import concourse.tile as tile
    from concourse import mybir

    BF = mybir.dt.bfloat16
    F32 = mybir.dt.float32
    OP = mybir.AluOpType
    AX = mybir.AxisListType.X
    ACTF = mybir.ActivationFunctionType

    nc = bacc.Bacc("TRN2", target_bir_lowering=False, debug=False,
                   num_devices=N_CORES)

    cd = nc.dram_tensor("cd", [3 * NBLK, 128, FREE], F32, kind="ExternalInput")
    ridx = nc.dram_tensor("ridx", [128, NBLK * NJ], BF, kind="ExternalInput")
    qidx = nc.dram_tensor("qidx", [128, NBLK * NJ], BF, kind="ExternalInput")
    iorep = nc.dram_tensor("iorep", [128, 128 * NJ], BF, kind="ExternalInput")
    io16r = nc.dram_tensor("io16r", [128, 16 * NJ], BF, kind="ExternalInput")
    wneg = nc.dram_tensor("wneg", [128, ND], BF, kind="ExternalInput")
    wpos = nc.dram_tensor("wpos", [128, ND], F32, kind="ExternalInput")
    coef = nc.dram_tensor("coef", [128, 2 * ND], F32, kind="ExternalInput")
    pf = nc.dram_tensor("pf", [128, 48], F32, kind="ExternalOutput")
    pe = nc.dram_tensor("pe", [128, 2], F32, kind="ExternalOutput")

    with tile.TileContext(nc) as tc:
        with nc.allow_low_precision("bf16 pipeline validated against fp32 reference"), \
             tc.tile_pool(name="singles", bufs=1) as singles, \
             tc.tile_pool(name="cdf", bufs=3) as cdf_pool, \
             tc.tile_pool(name="cdb", bufs=3) as cdb_pool, \
             tc.tile_pool(name="tmp", bufs=3) as tmp_pool, \
             tc.tile_pool(name="hh", bufs=3) as h_pool, \
             tc.tile_pool(name="ff", bufs=2) as f_pool, \
             tc.tile_pool(name="qg", bufs=2) as qg_pool, \
             tc.tile_pool(name="mm", bufs=2) as m_pool, \
             tc.tile_pool(name="psum", bufs=1, space="PSUM") as psum_pool:

            s_ridx = singles.tile([128, NBLK * NJ], BF)
            nc.sync.dma_start(out=s_ridx, in_=ridx[:, :])
            s_qidx = singles.tile([128, NBLK * NJ], BF)
            nc.sync.dma_start(out=s_qidx, in_=qidx[:, :])
            s_iorep = singles.tile([128, 128 * NJ], BF)
            nc.sync.dma_start(out=s_iorep, in_=iorep[:, :])
            s_io16r = singles.tile([128, 16 * NJ], BF)
            nc.sync.dma_start(out=s_io16r, in_=io16r[:, :])
            s_wneg = singles.tile([128, ND], BF)
            nc.sync.dma_start(out=s_wneg, in_=wneg[:, :])
            s_wpos = singles.tile([128, ND], F32)
            nc.sync.dma_start(out=s_wpos, in_=wpos[:, :])
            s_coef = singles.tile([128, 2 * ND], F32)
            nc.sync.dma_start(out=s_coef, in_=coef[:, :])

            # ---- e_pa partial: sum(coeffs * w) over this core's 256 atoms
            etmp = singles.tile([128, 2 * ND], F32)
            nc.vector.tensor_tensor(
                out=etmp[:].rearrange("p (a k) -> p a k", k=ND),
                in0=s_coef[:].rearrange("p (a k) -> p a k", k=ND),
                in1=s_wpos[:].unsqueeze(1).broadcast_to((128, 2, ND)),
                op=OP.mult)
            pe_sb = singles.tile([128, 2], F32)
            nc.vector.tensor_reduce(
                out=pe_sb, in_=etmp[:].rearrange("p (a k) -> p a k", k=ND),
                axis=AX, op=OP.add)
            nc.sync.dma_start(out=pe[:, :], in_=pe_sb)

            # ---- main pipeline
            pfp = psum_pool.tile([128, 48], F32)

            for b in range(NBLK):
                fb = f_pool.tile([128, NJ * 3], BF)
                for c in range(3):
                    i = c * NBLK + b
                    cdf = cdf_pool.tile([128, FREE], F32)
                    nc.sync.dma_start(out=cdf, in_=cd[i, :, :])
                    cdb = cdb_pool.tile([128, FREE], BF)
                    nc.scalar.copy(out=cdb, in_=cdf)          # ACT cast
                    tmp = tmp_pool.tile([128, FREE], BF)
                    nc.vector.tensor_tensor(                   # DVE: * (-w)
                        out=tmp[:].rearrange("p (j k) -> p j k", k=ND),
                        in0=cdb[:].rearrange("p (j k) -> p j k", k=ND),
                        in1=s_wneg[:].unsqueeze(1).broadcast_to((128, NJ, ND)),
                        op=OP.mult)
                    t3 = tmp[:].rearrange("p (j k) -> p j k", k=ND)
                    h1 = h_pool.tile([128, NJ * 32], BF, tag="h1")
                    h1v = h1[:].rearrange("p (j k) -> p j k", k=32)
                    nc.gpsimd.tensor_tensor(                   # GPSIMD: 64->32
                        out=h1v, in0=t3[:, :, 0:32], in1=t3[:, :, 32:64],
                        op=OP.add)
                    fbv = fb[:].rearrange("p (j c) -> p j c", c=3)[:, :, c]
                    nc.vector.tensor_reduce(                   # DVE: 32->1
                        out=fbv,
                        in_=h1[:].rearrange("p (j k) -> p j k", k=32),
                        axis=AX, op=OP.add)

                # qmask, layout qm[p, q*NJ + j]: one 2x-mode TT per block
                qm = qg_pool.tile([128, 16 * NJ], BF, tag="qm")
                nc.vector.tensor_tensor(
                    out=qm[:].rearrange("p (q j) -> p q j", j=NJ),
                    in0=s_io16r[:].rearrange("p (q j) -> p q j", j=NJ),
                    in1=s_qidx[:, b * NJ:(b + 1) * NJ].unsqueeze(1)
                        .broadcast_to((128, 16, NJ)),
                    op=OP.is_equal)
                # g = qmask * f  (Khatri-Rao via stride-0 broadcasts)
                g = qg_pool.tile([128, NJ * 48], BF, tag="g")
                nc.gpsimd.tensor_tensor(
                    out=g[:].rearrange("p (j q c) -> p j q c", q=16, c=3),
                    in0=qm[:].rearrange("p (q j) -> p q j", j=NJ)
                        .transpose([0, 2, 1])
                        .unsqueeze(3).broadcast_to((128, NJ, 16, 3)),
                    in1=fb[:].rearrange("p (j c) -> p j c", c=3)
                        .unsqueeze(2).broadcast_to((128, NJ, 16, 3)),
                    op=OP.mult)
                # match, layout mt[p, r*NJ + j]: one 2x-mode TT per block
                mt = m_pool.tile([128, 128 * NJ], BF)
                nc.vector.tensor_tensor(
                    out=mt[:].rearrange("p (r j) -> p r j", j=NJ),
                    in0=s_iorep[:].rearrange("p (r j) -> p r j", j=NJ),
                    in1=s_ridx[:, b * NJ:(b + 1) * NJ].unsqueeze(1)
                        .broadcast_to((128, 128, NJ)),
                    op=OP.is_equal)
                # scatter: psum[r, q*3+c] += onehot_r.T @ g
                mtv = mt[:].rearrange("p (r j) -> p r j", j=NJ)
                for j in range(NJ):
                    nc.tensor.matmul(
                        pfp,
                        lhsT=mtv[:, :, j],
                        rhs=g[:, j * 48:(j + 1) * 48],
                        start=(b == 0 and j == 0),
                        stop=(b == NBLK - 1 and j == NJ - 1))

            pf_sb = singles.tile([128, 48], F32)
            nc.scalar.copy(out=pf_sb, in_=pfp)
            nc.sync.dma_start(out=pf[:, :], in_=pf_sb)

    nc.compile()
    return nc


def _get_nc():
    if "nc" not in _CACHE:
        _CACHE["nc"] = _build_nc()
    return _CACHE["nc"]


def _host_prep(inputs):
    coeffs = np.asarray(inputs["coeffs"])            # [1, 2048, 64] f32
    cd_full = np.asarray(inputs["coeffs_derivs"])    # [1, 3, P, 64] f32
    nei = np.asarray(inputs["neigh_atom_index"])     # [P] int32
    w = np.asarray(inputs["weight1"])                # [1, 64] f32

    iorep = np.ascontiguousarray(np.tile(
        np.repeat(np.arange(128, dtype=np.float32), NJ).astype(_BF16),
        (128, 1)))
    io16r = np.ascontiguousarray(np.tile(
        np.repeat(np.arange(16, dtype=np.float32), NJ).astype(_BF16),
        (128, 1)))
    wneg = np.ascontiguousarray(np.tile((-w[0]).astype(_BF16), (128, 1)))
    wpos = np.ascontiguousarray(np.tile(w[0].astype(np.float32), (128, 1)))

    atoms_per_core = NUM_ATOMS // N_CORES            # 256
    in_maps = []
    for m in range(N_CORES):
        sl = slice(m * PLOC, (m + 1) * PLOC)
        cdm = np.ascontiguousarray(cd_full[0, :, sl, :]).reshape(
            3 * NBLK, 128, FREE)
        nm = nei[sl].reshape(NBLK, 128, NJ)          # [b, part, j]
        r = np.ascontiguousarray(
            (nm % 128).astype(np.float32).astype(_BF16)
            .transpose(1, 0, 2).reshape(128, NBLK * NJ))
        q = np.ascontiguousarray(
            (nm // 128).astype(np.float32).astype(_BF16)
            .transpose(1, 0, 2).reshape(128, NBLK * NJ))
        cf = np.ascontiguousarray(
            coeffs[0, m * atoms_per_core:(m + 1) * atoms_per_core, :]
            .reshape(2, 128, ND).transpose(1, 0, 2).reshape(128, 2 * ND))
        in_maps.append(dict(cd=cdm, ridx=r, qidx=q, iorep=iorep,
                            io16r=io16r, wneg=wneg, wpos=wpos, coef=cf))
    return in_maps


def run(inputs, trace=False, trace_kwargs=None):
    """Run the kernel; returns ((e_pa, out_f), BassKernelResults)."""
    from concourse.bass_utils import run_bass_kernel_spmd

    nc = _get_nc()
    in_maps = _host_prep(inputs)
    res = run_bass_kernel_spmd(
        nc, in_maps, core_ids=list(range(N_CORES)), trace=trace,
        **(trace_kwargs or {}))

    pf = np.zeros((128, 48), np.float64)
    pe_total = 0.0
    for i in range(N_CORES):
        pf += res.results[i]["pf"].astype(np.float64)
        pe_total += float(res.results[i]["pe"].sum(dtype=np.float64))

    out_f = np.ascontiguousarray(
        pf.reshape(128, 16, 3).transpose(2, 1, 0).reshape(1, 3, NUM_ATOMS)
        .astype(np.float32))
    bias1 = np.asarray(inputs["bias1"]).astype(np.float32)
    e_pa = (np.array([pe_total / NUM_ATOMS], np.float32) + bias1).astype(
        np.float32)
    return (e_pa, out_f), res


def kernel(**inputs):
    (e_pa, out_f), _ = run(inputs, trace=False)
    return (e_pa, out_f)
